# revision 5
# baseline (speedup 1.0000x reference)
# Trainium2 Bass kernel: 2:4 structured activation pruning + Linear.
#
#   out = magnitude_prune_2of4(x.reshape(-1, 4096)) @ weight.T
#
# Sharding: data-parallel over the flattened token dim (16384 tokens ->
# 2048/core across 8 cores); weight replicated (host-transposed so the
# contraction dim lands on SBUF partitions). No collectives.
#
# Per-core pipeline, per 128-token tile (free dim split in 2 halves of 2048):
#   DMA x -> ACT |x| -> DVE pairwise min/max tree -> per-group-of-4 2nd-max
#   threshold (exact fp32) -> DVE drop-mask + predicated zero (in place)
#   -> PE 128x128 transposes (fp32, exact) -> ACT PSUM->SBUF copy w/ cast to
#   float32r -> PE matmul (float32r, full rate) accumulating over 32 d-chunks
#   -> ACT PSUM->SBUF -> DMA out.
import numpy as np

N_CORES = 8
BS, SEQ, D = 4, 4096, 4096
OUTF = 1024
TOK_TOTAL = BS * SEQ
TOK = TOK_TOTAL // N_CORES      # 2048 tokens per core
P = 128                         # SBUF partitions
NT = TOK // P                   # 16 token tiles per core
HALF = D // 2                   # 2048: free-dim half width
NCH = D // P                    # 32 d-chunks of 128
NCH_H = NCH // 2                # 16 d-chunks per half

_compiled = None


def _build():
    import concourse.tile as tile
    import concourse.mybir as mybir
    from concourse import bacc
    from concourse.masks import make_identity

    f32 = mybir.dt.float32
    f32r = mybir.dt.float32r
    Alu = mybir.AluOpType

    nc = bacc.Bacc("TRN2", target_bir_lowering=False, debug=False,
                   num_devices=N_CORES)
    xs_ap = nc.dram_tensor("xs", [TOK, D], f32, kind="ExternalInput").ap()
    wt_ap = nc.dram_tensor("wt", [D, OUTF], f32r, kind="ExternalInput").ap()
    o_ap = nc.dram_tensor("o", [TOK, OUTF], f32, kind="ExternalOutput").ap()

    with tile.TileContext(nc) as tc:
        with tc.tile_pool(name="wpool", bufs=1) as wpool, \
             tc.tile_pool(name="consts", bufs=1) as consts, \
             tc.tile_pool(name="xin", bufs=2) as xin, \
             tc.tile_pool(name="mwork", bufs=1) as mwork, \
             tc.tile_pool(name="xtp", bufs=2) as xtp, \
             tc.tile_pool(name="outp", bufs=4) as outp, \
             tc.tile_pool(name="pstr", bufs=2, space="PSUM") as pstr, \
             tc.tile_pool(name="pso", bufs=4, space="PSUM") as pso:

            # ---- constants ----
            # weight.T resident in SBUF: [d-in-chunk partitions, chunk, outf]
            w_sb = wpool.tile([P, NCH, OUTF], f32r)
            for c in range(NCH):
                nc.sync.dma_start(out=w_sb[:, c, :],
                                  in_=wt_ap[c * P:(c + 1) * P, :])
            ident = consts.tile([P, P], f32)
            make_identity(nc, ident)
            zeros = consts.tile([P, 1], f32)
            nc.vector.memset(zeros, 0.0)
            zeros_b = zeros.broadcast_to([P, HALF])

            for i in range(NT):
                # float32r-rounded transposed pruned activations, [d, tok]
                xspT = xtp.tile([P, NCH, P], f32r)
                for h in range(2):
                    xh = xin.tile([P, HALF], f32)
                    nc.sync.dma_start(
                        out=xh,
                        in_=xs_ap[i * P:(i + 1) * P, h * HALF:(h + 1) * HALF])
                    absx = mwork.tile([P, HALF], f32, tag="absx")
                    nc.scalar.activation(absx, xh,
                                         mybir.ActivationFunctionType.Abs)
                    # pairwise tree: thr = 2nd-largest |x| per group of 4
                    a2 = absx.rearrange("p (g two) -> p g two", two=2)
                    mx = mwork.tile([P, HALF // 2], f32, tag="mx")
                    mn = mwork.tile([P, HALF // 2], f32, tag="mn")
                    nc.vector.tensor_tensor(mx, a2[:, :, 0], a2[:, :, 1], Alu.max)
                    nc.vector.tensor_tensor(mn, a2[:, :, 0], a2[:, :, 1], Alu.min)
                    mx2 = mx.rearrange("p (g two) -> p g two", two=2)
                    mn2 = mn.rearrange("p (g two) -> p g two", two=2)
                    mm = mwork.tile([P, HALF // 4], f32, tag="mm")
                    nm = mwork.tile([P, HALF // 4], f32, tag="nm")
                    nc.vector.tensor_tensor(mm, mx2[:, :, 0], mx2[:, :, 1], Alu.min)
                    nc.vector.tensor_tensor(nm, mn2[:, :, 0], mn2[:, :, 1], Alu.max)
                    thr = mwork.tile([P, HALF // 4], f32, tag="thr")
                    nc.vector.tensor_tensor(thr, mm, nm, Alu.max)
                    # drop-mask: |x| < thr (strictly below the 2nd-largest),
                    # written in place over |x| (read stream leads the write)
                    thr_b = thr.unsqueeze(2).broadcast_to([P, HALF // 4, 4])
                    nc.vector.tensor_tensor(
                        absx.rearrange("p (g four) -> p g four", four=4),
                        absx.rearrange("p (g four) -> p g four", four=4),
                        thr_b, Alu.is_lt)
                    # zero the dropped elements in place
                    nc.vector.copy_predicated(xh, absx.bitcast(mybir.dt.int32),
                                              zeros_b)
                    # transpose 16 chunks of [128,128] via PE, 4 per PSUM bank
                    for b in range(NCH_H // 4):
                        ptr = pstr.tile([P, 4 * P], f32)
                        for k in range(4):
                            cc = 4 * b + k
                            nc.tensor.transpose(ptr[:, k * P:(k + 1) * P],
                                                xh[:, cc * P:(cc + 1) * P],
                                                ident)
                        nc.scalar.copy(xspT[:, h * NCH_H + 4 * b:
                                            h * NCH_H + 4 * b + 4, :], ptr)
                # matmul: psum[tok, outf-half] += xspT[c].T @ wT[c]
                for n in range(2):
                    pout = pso.tile([P, OUTF // 2], f32)
                    for c in range(NCH):
                        nc.tensor.matmul(pout,
                                         xspT[:, c, :],
                                         w_sb[:, c, n * 512:(n + 1) * 512],
                                         start=(c == 0), stop=(c == NCH - 1))
                    osb = outp.tile([P, OUTF // 2], f32)
                    nc.scalar.copy(osb, pout)
                    nc.sync.dma_start(
                        out=o_ap[i * P:(i + 1) * P, n * 512:(n + 1) * 512],
                        in_=osb)
    nc.compile()
    return nc


def _get_compiled():
    global _compiled
    if _compiled is None:
        _compiled = _build()
    return _compiled


def _fix_ties(x_flat):
    # The device keeps elements with |x| >= (2nd-largest |x| of the group).
    # On an exact fp32 tie |2nd|==|3rd| that keeps 3 elements, while the
    # reference (top_k, stable) keeps the lower-indexed 2. Pre-zero the
    # reference-dropped elements of tied groups so the device agrees; the
    # zeroed elements are dropped either way, so values are unaffected.
    g = np.abs(x_flat.reshape(-1, 4))
    m1 = np.maximum(g[:, 0], g[:, 1]); n1 = np.minimum(g[:, 0], g[:, 1])
    m2 = np.maximum(g[:, 2], g[:, 3]); n2 = np.minimum(g[:, 2], g[:, 3])
    thr = np.maximum(np.minimum(m1, m2), np.maximum(n1, n2))
    third = np.minimum(np.minimum(m1, m2), np.maximum(n1, n2))
    tied = np.flatnonzero(thr == third)
    if len(tied) == 0:
        return x_flat
    x_flat = x_flat.copy()
    gv = x_flat.reshape(-1, 4)
    for t in tied:
        row = gv[t]
        order = np.argsort(-np.abs(row), kind="stable")
        row[order[2:]] = 0.0
    return x_flat


def kernel(x: np.ndarray, weight: np.ndarray) -> np.ndarray:
    from concourse.bass_utils import run_bass_kernel_spmd

    nc = _get_compiled()
    x_flat = np.ascontiguousarray(x.reshape(TOK_TOTAL, D), dtype=np.float32)
    x_flat = _fix_ties(x_flat)
    wt = np.ascontiguousarray(weight.T, dtype=np.float32)
    in_maps = [{"xs": x_flat[c * TOK:(c + 1) * TOK], "wt": wt}
               for c in range(N_CORES)]
    res = run_bass_kernel_spmd(nc, in_maps, core_ids=list(range(N_CORES)))
    out = np.concatenate([res.results[c]["o"] for c in range(N_CORES)], axis=0)
    return out.reshape(BS, SEQ, OUTF)


# revision 9
# speedup vs baseline: 1.1051x; 1.1051x over previous
# Trainium2 Bass kernel: 2:4 structured activation pruning + Linear.
#
#   out = magnitude_prune_2of4(x.reshape(-1, 4096)) @ weight.T
#
# Sharding: data-parallel over the flattened token dim (16384 tokens ->
# 2048/core across 8 cores); weight replicated (host-transposed so the
# contraction dim lands on SBUF partitions). No collectives.
#
# Per-core pipeline, per 128-token tile (free dim split in 2 halves of 2048):
#   DMA x -> ACT |x| -> DVE pairwise min/max tree -> per-group-of-4 2nd-max
#   threshold (exact fp32) -> DVE drop-mask + predicated zero (in place)
#   -> PE 128x128 transposes (fp32, exact) -> ACT PSUM->SBUF copy w/ cast to
#   float32r -> PE matmul (float32r, full rate) accumulating over 32 d-chunks
#   -> ACT PSUM->SBUF -> DMA out.
import numpy as np

N_CORES = 8
BS, SEQ, D = 4, 4096, 4096
OUTF = 1024
TOK_TOTAL = BS * SEQ
TOK = TOK_TOTAL // N_CORES      # 2048 tokens per core
P = 128                         # SBUF partitions
NT = TOK // P                   # 16 token tiles per core
HALF = D // 2                   # 2048: free-dim half width
NCH = D // P                    # 32 d-chunks of 128
NCH_H = NCH // 2                # 16 d-chunks per half

_compiled = None


def _build():
    import concourse.tile as tile
    import concourse.mybir as mybir
    from concourse import bacc
    from concourse.masks import make_identity

    f32 = mybir.dt.float32
    f32r = mybir.dt.float32r
    Alu = mybir.AluOpType

    nc = bacc.Bacc("TRN2", target_bir_lowering=False, debug=False,
                   num_devices=N_CORES)
    xs_ap = nc.dram_tensor("xs", [TOK, D], f32, kind="ExternalInput").ap()
    wt_ap = nc.dram_tensor("wt", [D, OUTF], f32r, kind="ExternalInput").ap()
    o_ap = nc.dram_tensor("o", [TOK, OUTF], f32, kind="ExternalOutput").ap()

    with tile.TileContext(nc) as tc:
        with tc.tile_pool(name="wpool", bufs=1) as wpool, \
             tc.tile_pool(name="consts", bufs=1) as consts, \
             tc.tile_pool(name="xin", bufs=2) as xin, \
             tc.tile_pool(name="mwork", bufs=1) as mwork, \
             tc.tile_pool(name="xtp", bufs=2) as xtp, \
             tc.tile_pool(name="outp", bufs=2) as outp, \
             tc.tile_pool(name="pstr", bufs=2, space="PSUM") as pstr, \
             tc.tile_pool(name="pso", bufs=4, space="PSUM") as pso:

            # ---- constants ----
            # weight.T resident in SBUF: [d-in-chunk partitions, chunk, outf]
            w_sb = wpool.tile([P, NCH, OUTF], f32r)
            for c in range(NCH):
                nc.sync.dma_start(out=w_sb[:, c, :],
                                  in_=wt_ap[c * P:(c + 1) * P, :])
            ident = consts.tile([P, P], f32)
            make_identity(nc, ident)
            zeros = consts.tile([P, 1], f32)
            nc.vector.memset(zeros, 0.0)
            zeros_b = zeros.broadcast_to([P, HALF])

            for i in range(NT):
                # float32r-rounded transposed pruned activations, [d, tok]
                xspT = xtp.tile([P, NCH, P], f32r)
                for h in range(2):
                    xh = xin.tile([P, HALF], f32)
                    nc.sync.dma_start(
                        out=xh,
                        in_=xs_ap[i * P:(i + 1) * P, h * HALF:(h + 1) * HALF])
                    absx = mwork.tile([P, HALF], f32, tag="absx", bufs=2)
                    nc.scalar.activation(absx, xh,
                                         mybir.ActivationFunctionType.Abs)
                    # pairwise tree: thr = 2nd-largest |x| per group of 4
                    a2 = absx.rearrange("p (g two) -> p g two", two=2)
                    mx = mwork.tile([P, HALF // 2], f32, tag="mx")
                    mn = mwork.tile([P, HALF // 2], f32, tag="mn")
                    nc.vector.tensor_tensor(mx, a2[:, :, 0], a2[:, :, 1], Alu.max)
                    nc.vector.tensor_tensor(mn, a2[:, :, 0], a2[:, :, 1], Alu.min)
                    # compact in place: writes trail the strided reads
                    mx2 = mx.rearrange("p (g two) -> p g two", two=2)
                    mn2 = mn.rearrange("p (g two) -> p g two", two=2)
                    mm = mx[:, :HALF // 4]
                    nm = mn[:, :HALF // 4]
                    nc.vector.tensor_tensor(mm, mx2[:, :, 0], mx2[:, :, 1], Alu.min)
                    nc.vector.tensor_tensor(nm, mn2[:, :, 0], mn2[:, :, 1], Alu.max)
                    thr = mm
                    nc.vector.tensor_tensor(thr, mm, nm, Alu.max)
                    # drop-mask: |x| < thr (strictly below the 2nd-largest),
                    # written in place over |x| (read stream leads the write)
                    thr_b = thr.unsqueeze(2).broadcast_to([P, HALF // 4, 4])
                    nc.vector.tensor_tensor(
                        absx.rearrange("p (g four) -> p g four", four=4),
                        absx.rearrange("p (g four) -> p g four", four=4),
                        thr_b, Alu.is_lt)
                    # zero the dropped elements in place
                    nc.vector.copy_predicated(xh, absx.bitcast(mybir.dt.int32),
                                              zeros_b)
                    # transpose 16 chunks of [128,128] via PE, 4 per PSUM bank
                    for b in range(NCH_H // 4):
                        ptr = pstr.tile([P, 4 * P], f32)
                        for k in range(4):
                            cc = 4 * b + k
                            nc.tensor.transpose(ptr[:, k * P:(k + 1) * P],
                                                xh[:, cc * P:(cc + 1) * P],
                                                ident)
                        nc.scalar.copy(xspT[:, h * NCH_H + 4 * b:
                                            h * NCH_H + 4 * b + 4, :], ptr)
                # matmul: psum[tok, outf-half] += xspT[c].T @ wT[c]
                for n in range(2):
                    pout = pso.tile([P, OUTF // 2], f32)
                    for c in range(NCH):
                        nc.tensor.matmul(pout,
                                         xspT[:, c, :],
                                         w_sb[:, c, n * 512:(n + 1) * 512],
                                         start=(c == 0), stop=(c == NCH - 1))
                    osb = outp.tile([P, OUTF // 2], f32)
                    nc.scalar.copy(osb, pout)
                    nc.sync.dma_start(
                        out=o_ap[i * P:(i + 1) * P, n * 512:(n + 1) * 512],
                        in_=osb)
    nc.compile()
    return nc


def _get_compiled():
    global _compiled
    if _compiled is None:
        _compiled = _build()
    return _compiled


def _fix_ties(x_flat):
    # The device keeps elements with |x| >= (2nd-largest |x| of the group).
    # On an exact fp32 tie |2nd|==|3rd| that keeps 3 elements, while the
    # reference (top_k, stable) keeps the lower-indexed 2. Pre-zero the
    # reference-dropped elements of tied groups so the device agrees; the
    # zeroed elements are dropped either way, so values are unaffected.
    g = np.abs(x_flat.reshape(-1, 4))
    m1 = np.maximum(g[:, 0], g[:, 1]); n1 = np.minimum(g[:, 0], g[:, 1])
    m2 = np.maximum(g[:, 2], g[:, 3]); n2 = np.minimum(g[:, 2], g[:, 3])
    thr = np.maximum(np.minimum(m1, m2), np.maximum(n1, n2))
    third = np.minimum(np.minimum(m1, m2), np.maximum(n1, n2))
    tied = np.flatnonzero(thr == third)
    if len(tied) == 0:
        return x_flat
    x_flat = x_flat.copy()
    gv = x_flat.reshape(-1, 4)
    for t in tied:
        row = gv[t]
        order = np.argsort(-np.abs(row), kind="stable")
        row[order[2:]] = 0.0
    return x_flat


def kernel(x: np.ndarray, weight: np.ndarray) -> np.ndarray:
    from concourse.bass_utils import run_bass_kernel_spmd

    nc = _get_compiled()
    x_flat = np.ascontiguousarray(x.reshape(TOK_TOTAL, D), dtype=np.float32)
    x_flat = _fix_ties(x_flat)
    wt = np.ascontiguousarray(weight.T, dtype=np.float32)
    in_maps = [{"xs": x_flat[c * TOK:(c + 1) * TOK], "wt": wt}
               for c in range(N_CORES)]
    res = run_bass_kernel_spmd(nc, in_maps, core_ids=list(range(N_CORES)))
    out = np.concatenate([res.results[c]["o"] for c in range(N_CORES)], axis=0)
    return out.reshape(BS, SEQ, OUTF)
